# revision 12
# baseline (speedup 1.0000x reference)
"""Causal RoPE attention (B=4, T=2048, D=2048, H=16, Dh=128) on 8 trn2 cores.

Sharding (no collectives): core c handles batch b = c//2 and query-block
parity p = c%2.  T is split into 8 blocks of 256 queries; parity p owns
blocks {2j+p : j=0..3} ("slots").  Slot j attends k-blocks 0..4j+3 (128 keys
each) — uniform across cores; host-supplied masks kill the out-of-range /
future keys, so the SPMD program is shape-identical on every core.

Fully fused single-pass design: x^T stays resident in SBUF (bf16); for each
head, K/Q (with RoPE) and V are projected straight into SBUF, attention runs
out of SBUF, and per-head attention outputs accumulate in SBUF until the
final output projection.  No DRAM round-trips for intermediates.  All matmul
operands are bf16 (fp32 PSUM accumulation).
"""

import sys

sys.path.insert(0, "/opt/trn_rl_repo")

import numpy as np

D = 2048
T = 2048
H = 16
DH = 128
B = 4
NSLOT = 4          # query slots per core
QW = 256           # queries per slot
QCOLS = NSLOT * QW # 1024 query columns per core
SCALE = 1.0 / np.sqrt(128.0)
MASKV = -1.0e5

_compiled = {}


def _build_nc():
    import concourse.bacc as bacc
    from concourse import mybir
    from concourse.tile import TileContext

    F32 = mybir.dt.float32
    BF16 = mybir.dt.bfloat16
    EXP = mybir.ActivationFunctionType.Exp

    nc = bacc.Bacc(trn_type="TRN2")

    xt_d = nc.dram_tensor("xt", [128, 16, T], BF16, kind="ExternalInput")
    xq_d = nc.dram_tensor("xq", [128, 16, QCOLS], BF16, kind="ExternalInput")
    wk_d = nc.dram_tensor("wk", [H, 128, 16, DH], BF16, kind="ExternalInput")
    wq_d = nc.dram_tensor("wq", [H, 128, 16, DH], BF16, kind="ExternalInput")
    wv_d = nc.dram_tensor("wv", [H, 128, 16, DH], BF16, kind="ExternalInput")
    wo_d = nc.dram_tensor("wo", [H, 128, D], BF16, kind="ExternalInput")
    cosk_d = nc.dram_tensor("cosk", [128, T], BF16, kind="ExternalInput")
    sink_d = nc.dram_tensor("sink", [128, T], BF16, kind="ExternalInput")
    cosq_d = nc.dram_tensor("cosq", [128, QCOLS], BF16, kind="ExternalInput")
    sinq_d = nc.dram_tensor("sinq", [128, QCOLS], BF16, kind="ExternalInput")
    mask_d = nc.dram_tensor("mask", [4, 128, QW], F32, kind="ExternalInput")
    ones_d = nc.dram_tensor("ones", [128, 2], BF16, kind="ExternalInput")
    ident_d = nc.dram_tensor("ident", [128, 128], BF16, kind="ExternalInput")
    out_d = nc.dram_tensor("out", [QCOLS, D], F32, kind="ExternalOutput")

    with TileContext(nc) as tc:
        with tc.tile_pool(name="persist", bufs=1) as persist:
            xt = persist.tile([128, 16, T], BF16, name="xt")
            for dc in range(16):
                nc.sync.dma_start(xt[:, dc, :], xt_d[:, dc, :])
            xq = persist.tile([128, 16, QCOLS], BF16, name="xq")
            nc.sync.dma_start(xq[:], xq_d[:])
            ck = persist.tile([128, T], BF16, name="ck")
            sk = persist.tile([128, T], BF16, name="sk")
            cq = persist.tile([128, QCOLS], BF16, name="cq")
            sq = persist.tile([128, QCOLS], BF16, name="sq")
            nc.sync.dma_start(ck[:], cosk_d[:])
            nc.sync.dma_start(sk[:], sink_d[:])
            nc.sync.dma_start(cq[:], cosq_d[:])
            nc.sync.dma_start(sq[:], sinq_d[:])
            msk_sb = []
            for m in range(4):
                mt = persist.tile([128, QW], F32, name=f"msk{m}")
                nc.sync.dma_start(mt[:], mask_d[m])
                msk_sb.append(mt)
            ones_sb = persist.tile([128, 2], BF16, name="ones")
            nc.sync.dma_start(ones_sb[:], ones_d[:])
            ident = persist.tile([128, 128], BF16, name="ident")
            nc.sync.dma_start(ident[:], ident_d[:])
            # normalized attention outputs, [dh, q] per head; live to the end
            attn_sb = [
                persist.tile([128, QCOLS], BF16, name=f"attn{h}")
                for h in range(H)
            ]

            with tc.tile_pool(name="wts", bufs=1) as wts, \
                 tc.tile_pool(name="kvq", bufs=1) as kvq, \
                 tc.tile_pool(name="ev", bufs=3) as ev, \
                 tc.tile_pool(name="sep", bufs=3) as sep, \
                 tc.tile_pool(name="nrm", bufs=2) as nrm, \
                 tc.tile_pool(name="pp", bufs=2, space="PSUM") as ppp, \
                 tc.tile_pool(name="pst", bufs=3, space="PSUM") as pstp, \
                 tc.tile_pool(name="pau", bufs=2, space="PSUM") as paup, \
                 tc.tile_pool(name="pdn", bufs=1, space="PSUM") as pdnp:
                for h in range(H):
                    # ---- weights for this head ----------------------------
                    wkt = wts.tile([128, 16, DH], BF16, tag="wk", name=f"wk{h}")
                    nc.sync.dma_start(wkt[:], wk_d[h])
                    wqt = wts.tile([128, 16, DH], BF16, tag="wq", name=f"wq{h}")
                    nc.sync.dma_start(wqt[:], wq_d[h])
                    wvt = wts.tile([128, 16, DH], BF16, tag="wv", name=f"wv{h}")
                    nc.sync.dma_start(wvt[:], wv_d[h])

                    # ---- K projection + RoPE -> KT (SBUF) -----------------
                    KT = kvq.tile([128, T], BF16, tag="kt", name=f"kt{h}")
                    for c4 in range(4):
                        sl = slice(c4 * 512, (c4 + 1) * 512)
                        pk = ppp.tile([128, 512], F32, tag="pp")
                        for dc in range(16):
                            nc.tensor.matmul(pk[:], wkt[:, dc, :],
                                             xt[:, dc, sl],
                                             start=(dc == 0), stop=(dc == 15))
                        t1 = ev.tile([128, 512], F32, tag="t1")
                        t2 = ev.tile([128, 512], F32, tag="t2")
                        nc.vector.tensor_mul(t1[:], pk[:], ck[:, sl])
                        nc.vector.tensor_mul(t2[0:64, :], pk[64:128, :],
                                             sk[0:64, sl])
                        nc.vector.tensor_mul(t2[64:128, :], pk[0:64, :],
                                             sk[64:128, sl])
                        nc.vector.tensor_add(KT[:, sl], t1[:], t2[:])

                    # ---- Q projection + RoPE -> QT ------------------------
                    QT = kvq.tile([128, QCOLS], BF16, tag="qt", name=f"qt{h}")
                    for j2 in range(2):
                        qsl = slice(j2 * 512, (j2 + 1) * 512)
                        pq = ppp.tile([128, 512], F32, tag="pp")
                        for dc in range(16):
                            nc.tensor.matmul(pq[:], wqt[:, dc, :],
                                             xq[:, dc, qsl],
                                             start=(dc == 0), stop=(dc == 15))
                        t1 = ev.tile([128, 512], F32, tag="t1")
                        t2 = ev.tile([128, 512], F32, tag="t2")
                        nc.vector.tensor_mul(t1[:], pq[:], cq[:, qsl])
                        nc.vector.tensor_mul(t2[0:64, :], pq[64:128, :],
                                             sq[0:64, qsl])
                        nc.vector.tensor_mul(t2[64:128, :], pq[0:64, :],
                                             sq[64:128, qsl])
                        nc.vector.tensor_add(QT[:, qsl], t1[:], t2[:])

                    # ---- V projection -> VT -> V (SBUF) -------------------
                    # VT[dh, t] projected like K (wide moving operand keeps
                    # LDWEIGHTS hidden), then PE-transposed per 128-block.
                    VT = kvq.tile([128, T], BF16, tag="vt", name=f"vt{h}")
                    for c4 in range(4):
                        sl = slice(c4 * 512, (c4 + 1) * 512)
                        pv = ppp.tile([128, 512], F32, tag="pp")
                        for dc in range(16):
                            nc.tensor.matmul(pv[:], wvt[:, dc, :],
                                             xt[:, dc, sl],
                                             start=(dc == 0), stop=(dc == 15))
                        nc.vector.tensor_copy(VT[:, sl], pv[:])
                    V = kvq.tile([128, 16, DH], BF16, tag="v", name=f"v{h}")
                    for tb in range(16):
                        tp = ppp.tile([128, DH], BF16, tag="pp")
                        nc.tensor.transpose(
                            tp[:], VT[:, tb * 128:(tb + 1) * 128], ident[:])
                        nc.vector.tensor_copy(V[:, tb, :], tp[:])

                    # ---- causal attention ---------------------------------
                    for j in range(NSLOT):
                        nk = 4 * j + 4
                        qsl = slice(j * QW, (j + 1) * QW)
                        au = paup.tile([128, QW], F32, tag="au",
                                       name=f"au{h}_{j}")
                        dnm = pdnp.tile([2, QW], F32, tag="dn",
                                        name=f"dn{h}_{j}")
                        for kb in range(nk):
                            st = pstp.tile([128, QW], F32, tag="st")
                            nc.tensor.matmul(
                                st[:], KT[:, kb * 128:(kb + 1) * 128],
                                QT[:, qsl], start=True, stop=True)
                            se = sep.tile([128, QW], BF16, tag="se")
                            if kb >= nk - 4:
                                sm = sep.tile([128, QW], F32, tag="sm")
                                nc.vector.tensor_add(
                                    sm[:], st[:], msk_sb[kb - (nk - 4)][:])
                                nc.scalar.activation(se[:], sm[:], EXP,
                                                     scale=SCALE)
                            else:
                                nc.scalar.activation(se[:], st[:], EXP,
                                                     scale=SCALE)
                            nc.tensor.matmul(au[:], V[:, kb, :], se[:],
                                             start=(kb == 0),
                                             stop=(kb == nk - 1))
                            nc.tensor.matmul(dnm[:], ones_sb[:], se[:],
                                             start=(kb == 0),
                                             stop=(kb == nk - 1))
                        rec = nrm.tile([1, QW], F32, tag="rec")
                        nc.vector.reciprocal(rec[:], dnm[:1, :])
                        rbc = nrm.tile([128, QW], F32, tag="rbc")
                        nc.gpsimd.partition_broadcast(rbc[:], rec[:])
                        nc.vector.tensor_mul(attn_sb[h][:, qsl], au[:], rbc[:])

            # ---------------- output projection ----------------------------
            # out[q, o] = sum_h sum_dh attn[h][dh, q] * wo[h][dh, o]
            with tc.tile_pool(name="wop", bufs=3) as wop, \
                 tc.tile_pool(name="evo", bufs=4) as evo, \
                 tc.tile_pool(name="po", bufs=8, space="PSUM") as pop:
                for oc in range(4):       # out col chunks of 512
                    osl = slice(oc * 512, (oc + 1) * 512)
                    po = [pop.tile([128, 512], F32, tag="po",
                                   name=f"po{oc}_{rb}")
                          for rb in range(8)]
                    for h in range(H):
                        wt = wop.tile([128, 512], BF16, tag="wo",
                                      name=f"wo{oc}_{h}")
                        nc.sync.dma_start(wt[:], wo_d[h][:, osl])
                        for rb in range(8):
                            nc.tensor.matmul(
                                po[rb][:],
                                attn_sb[h][:, rb * 128:(rb + 1) * 128], wt[:],
                                start=(h == 0), stop=(h == H - 1))
                    for rb in range(8):
                        oo = evo.tile([128, 512], F32, tag="oo")
                        if rb % 2 == 0:
                            nc.vector.tensor_copy(oo[:], po[rb][:])
                        else:
                            nc.scalar.copy(oo[:], po[rb][:])
                        nc.sync.dma_start(out_d[rb * 128:(rb + 1) * 128, osl],
                                          oo[:])

    nc.compile()
    return nc


def _host_prep(x, rope_cos, rope_sin, w_q, w_k, w_v, w_o):
    import ml_dtypes

    BF = ml_dtypes.bfloat16
    f32 = np.float32
    x = np.asarray(x, dtype=f32)

    cosT = np.ascontiguousarray(rope_cos.T, dtype=f32)   # [128, T]
    sinT = np.ascontiguousarray(rope_sin.T, dtype=f32)
    sinTs = sinT.copy()
    sinTs[:64] = -sinTs[:64]
    ck = cosT.astype(BF)
    sk = sinTs.astype(BF)

    def byhead(w):
        # w is [out, in]; wT[d, o] = w[o, d]
        # result[h, p, dc, c] = wT[dc*128 + p, h*128 + c]
        wT = np.asarray(w, dtype=f32).T
        return np.ascontiguousarray(
            wT.reshape(16, 128, H, DH).transpose(2, 1, 0, 3)).astype(BF)

    wk = byhead(w_k)
    wq = byhead(w_q)
    wv = byhead(w_v)
    # wo rows by head: [h, dh, o]
    woT = np.asarray(w_o, dtype=f32).T
    wo = np.ascontiguousarray(woT.reshape(H, DH, D)).astype(BF)
    ones = np.ones((128, 2), dtype=BF)
    ident = np.eye(128, dtype=f32).astype(BF)

    # masks for the last four k-blocks of each slot (parity-dependent)
    ki = np.arange(128)[:, None]
    qi = np.arange(QW)[None, :]
    tri0 = np.where(qi >= ki, 0.0, MASKV).astype(f32)
    tri1 = np.where(qi >= ki + 128, 0.0, MASKV).astype(f32)
    neg = np.full((128, QW), MASKV, dtype=f32)
    zero = np.zeros((128, QW), dtype=f32)
    masks = {
        0: np.stack([tri0, tri1, neg, neg]),
        1: np.stack([zero, zero, tri0, tri1]),
    }

    qrows = {}
    for p in range(2):
        blocks = [2 * j + p for j in range(NSLOT)]
        qrows[p] = np.concatenate(
            [np.arange(b * QW, (b + 1) * QW) for b in blocks])

    in_maps = []
    for c in range(8):
        b, p = c // 2, c % 2
        xT = np.ascontiguousarray(x[b].T)              # [D, T]
        xt = np.ascontiguousarray(
            xT.reshape(16, 128, T).transpose(1, 0, 2)).astype(BF)
        xqc = np.ascontiguousarray(
            xT[:, qrows[p]].reshape(16, 128, QCOLS).transpose(1, 0, 2)
        ).astype(BF)
        in_maps.append({
            "xt": xt,
            "xq": xqc,
            "wk": wk,
            "wq": wq,
            "wv": wv,
            "wo": wo,
            "cosk": ck,
            "sink": sk,
            "cosq": np.ascontiguousarray(ck[:, qrows[p]]),
            "sinq": np.ascontiguousarray(sk[:, qrows[p]]),
            "mask": masks[p],
            "ones": ones,
            "ident": ident,
        })
    return in_maps, qrows


def kernel(x, rope_cos, rope_sin, w_q, w_k, w_v, w_o):
    from concourse.bass_utils import run_bass_kernel_spmd

    if "nc" not in _compiled:
        _compiled["nc"] = _build_nc()
    nc = _compiled["nc"]

    in_maps, qrows = _host_prep(np.asarray(x), np.asarray(rope_cos),
                                np.asarray(rope_sin), np.asarray(w_q),
                                np.asarray(w_k), np.asarray(w_v),
                                np.asarray(w_o))
    res = run_bass_kernel_spmd(nc, in_maps, core_ids=list(range(8)))
    out = np.empty((B, T, D), dtype=np.float32)
    for c in range(8):
        b, p = c // 2, c % 2
        out[b, qrows[p], :] = res.results[c]["out"]
    return out
